# revision 5
# baseline (speedup 1.0000x reference)
"""MiniBatchDiscrimination Trainium2 kernel, v2 (triangular / symmetric).

Math (per reference):
    act = (x @ W).reshape(B, K, D)              # B=256, K=100, D=50
    l1[i,k,j] = sum_d |act[i,k,d] - act[j,k,d]|
    features[i,k] = sum_j exp(-l1[i,k,j])
    out = concat([x, features], axis=1)

Sharding: kernels K are sharded across 8 cores (13 per core, padded to
104).  Each core does the full pairwise work for its 13 kernels.

v2 exploits l1 symmetry: only pairs with j >= 8*floor(i/8) are computed.
For i-octet g (i = 8g+2b+h, b<4, h<2) the j-range is [8g, 256).  All
phase-B tiles use absolute-j columns: diff tiles are [rows, 2, 256]
with only cols [8g, 256) written per octet; the PE matmul streams the
[.., :, 8g:] slice.  l1 identity: |y| = 2 relu(y) - y, so
l1 = 2 R' - A_j + A_i with R'[i,j] = sum_d relu(a_jd - a_id) and
A[r, j] = sum_d act[(r,d), j].

Per-octet pipeline:
  - relu diffs via DVE tensor_scalar (subtract, max vs 0; bf16 4x mode)
    and ScalarE activation (Relu, bias=-a_i) for a tuned subset of
    (chunk, octet) units to balance the two engines.
  - PE reduce: stationary 2.0 block-diag maps (k,d)-rows to r; the
    chunk-5 moving tile carries 13 constant A-rows (written once) with
    -I13 stationary, giving pl1 = 2R' - A_j with no extra matmuls.
  - exp: ScalarE Exp, scale=-1, per-partition bias -A_i
    -> jt = exp(-l1) exactly (diagonal exactly 1).
  - own-sum: one DVE tensor_reduce per octet -> feat_own[:, 2g:2g+2].
  - mirror-sum: ones-stationary PE matmul accumulating
    sum_{i in octet} jt[i, j] into persistent PSUM feat_mir[r, j] for
    j >= 8g+8 (within-octet pairs are covered by the own-sums).
Host: feature[m, r] = own[m, r] + mir[r, m]; concat with x (exact).
"""

import numpy as np
import ml_dtypes
from contextlib import ExitStack

import concourse.bass as bass
import concourse.bacc as bacc
import concourse.tile as tile
from concourse import mybir
from concourse.ap import AP
from concourse.bass_utils import run_bass_kernel_spmd

B = 256          # batch
IN_D = 1024      # input dim
NK = 13          # kernels per core (8*13 = 104 >= 100)
DK = 50          # dim per kernel
COLS = NK * DK   # 650 act_T rows per core
N_CORES = 8
CHUNKS = [(0, 128), (128, 128), (256, 128), (384, 128), (512, 128), (640, 10)]
NCH = len(CHUNKS)
AOFF = 32        # partition offset of the A-rows in the combined tiles
SW = 1088        # s-pack width

F32 = mybir.dt.float32
BF16 = mybir.dt.bfloat16


def _act_unit(t, g, b4, h):
    """True if this diff unit runs on ScalarE (Activation engine).

    Per-octet balance: ScalarE takes 11 of the 48 units each octet
    (unit costs: ScalarE ~185+0.833*JL ns vs DVE ~60+0.26*JL ns, and
    ScalarE also runs the two Exp instructions)."""
    return t == 0 or (t == 1 and 2 * b4 + h < 2)


def build_nc():
    nc = bacc.Bacc()
    xT_d = nc.declare_dram_parameter("xT", [IN_D, B], BF16, isOutput=False)
    w_d = nc.declare_dram_parameter("w", [IN_D, COLS], BF16, isOutput=False)
    s_d = nc.declare_dram_parameter("s", [128, SW], BF16, isOutput=False)
    fown_d = nc.declare_dram_parameter("fown", [128, 64], F32, isOutput=True)
    fmir_d = nc.declare_dram_parameter("fmir", [32, B], F32, isOutput=True)

    with ExitStack() as ctx:
        tc = ctx.enter_context(tile.TileContext(nc))
        const_pool = ctx.enter_context(tc.tile_pool(name="const", bufs=1))
        psum_a = ctx.enter_context(tc.tile_pool(name="psum_a", bufs=1, space="PSUM"))
        psum_b = ctx.enter_context(tc.tile_pool(name="psum_b", bufs=3, space="PSUM"))
        psum_m = ctx.enter_context(tc.tile_pool(name="psum_m", bufs=1, space="PSUM"))
        diff_pool = ctx.enter_context(tc.tile_pool(name="diff", bufs=8))
        jt_pool = ctx.enter_context(tc.tile_pool(name="jt", bufs=4))

        # ---- load inputs; W in two column halves so phase A's first
        # half (chunks 0-2) starts after ~1/2 of the W bytes arrive ----
        WSPLIT = 384
        xt_all = const_pool.tile([128, 8, B], BF16, tag="xt", name="xt_all")
        nc.sync.dma_start(
            out=xt_all[:],
            in_=AP(xT_d, 0, [[B, 128], [128 * B, 8], [1, B]]))
        xt_tiles = [xt_all[:, k, :] for k in range(8)]
        s_tile = const_pool.tile([128, SW], BF16, tag="s", name="s_tile")
        nc.sync.dma_start(out=s_tile[:], in_=s_d[:])
        wl_all = const_pool.tile([128, 8, WSPLIT], BF16, tag="wl",
                                 name="wl_all")
        for kh in range(2):
            nc.sync.dma_start(
                out=wl_all[:, 4 * kh:4 * kh + 4, :],
                in_=AP(w_d, 128 * COLS * 4 * kh,
                       [[COLS, 128], [128 * COLS, 4], [1, WSPLIT]]))
        w_lo = [wl_all[:, k, :] for k in range(8)]
        wh_all = const_pool.tile([128, 8, COLS - WSPLIT], BF16, tag="wh",
                                 name="wh_all")
        for kh in range(2):
            nc.sync.dma_start(
                out=wh_all[:, 4 * kh:4 * kh + 4, :],
                in_=AP(w_d, WSPLIT + 128 * COLS * 4 * kh,
                       [[COLS, 128], [128 * COLS, 4], [1, COLS - WSPLIT]]))
        w_hi = [wh_all[:, k, :] for k in range(8)]

        # ---- Phase A: act_T = W.T @ xT (k outer: overlap DMA/PE) ----
        act_bf = []    # bf16 tiles; act_bf[5] is the combined tile
        act_f32 = []   # fp32 upcasts (DVE tensor_scalar scalar operands)
        act_neg = {}   # negated fp32 (ScalarE relu bias)
        tb5 = const_pool.tile([AOFF + NK, B], BF16, tag="actb5", name="tb5")
        nc.gpsimd.memset(tb5[:], 0.0)
        # A-table: A = WS.T @ xT (WS = host-precomputed per-kernel column
        # sums of W, in the s-pack).  Needs only xT + s, so it runs while
        # the W halves are still in flight and warms up the PE.
        pA = psum_a.tile([32, B], F32, name="pA")
        for k in range(8):
            nc.tensor.matmul(
                pA[:],
                s_tile[0:128, 704 + 32 * k:704 + 32 * k + 32],
                xt_tiles[k][:],
                start=(k == 0),
                stop=(k == 7),
            )
        pa_tiles = {}
        for half in range(2):
            ts = range(3 * half, 3 * half + 3)
            for t in ts:
                pa_tiles[t] = psum_a.tile([CHUNKS[t][1], B], F32,
                                          tag=f"pa{t % 3}", name=f"pa{t}")
            for t in ts:
                for k in range(8):
                    mstart, msz = CHUNKS[t]
                    if half == 0:
                        wsl = w_lo[k][:, mstart:mstart + msz]
                    else:
                        wsl = w_hi[k][:, mstart - WSPLIT:mstart - WSPLIT + msz]
                    nc.tensor.matmul(
                        pa_tiles[t][:],
                        wsl,
                        xt_tiles[k][:],
                        start=(k == 0),
                        stop=(k == 7),
                    )
        for t, (mstart, msz) in enumerate(CHUNKS):
            pa = pa_tiles[t]
            if t < NCH - 1:
                tb = const_pool.tile([msz, B], BF16, tag=f"actb{t}",
                                     name=f"actb{t}")
                nc.vector.tensor_copy(tb[:], pa[:])
                src = tb[:]
                act_bf.append(tb)
            else:
                nc.vector.tensor_copy(tb5[0:10, :], pa[:])
                src = tb5[0:10, :]
                act_bf.append(tb5)
            tf = const_pool.tile([msz, B], F32, tag=f"actf{t}", name=f"actf{t}")
            nc.gpsimd.tensor_copy(tf[:], src)
            act_f32.append(tf)
            if any(_act_unit(t, g, b4, h) for g in range(32)
                   for b4 in range(4) for h in range(2)):
                tn = const_pool.tile([msz, B], F32, tag=f"actn{t}",
                                     name=f"actn{t}")
                nc.scalar.mul(tn[:], src, -1.0)
                act_neg[t] = tn

        # A values -> rows AOFF.. of tb5 (bf16)
        nc.vector.tensor_copy(tb5[AOFF:AOFF + NK, :], pA[0:NK, :])
        a_f32 = const_pool.tile([NK, B], F32, tag="a_f32", name="a_f32")
        nc.gpsimd.tensor_copy(a_f32[:], tb5[AOFF:AOFF + NK, :])

        # A_i layout: bias_lay[32b + r, 2g + h] = +A[r, 8g + 2b + h]
        # (added INTO pl1 by a rank-1 matmul; l1 = 2R' - A_j + A_i)
        bias_lay = const_pool.tile([128, 64], F32, tag="bias_lay",
                                   name="bias_lay")
        nc.gpsimd.memset(bias_lay[:], 0.0)
        for b4 in range(4):
            dst = bias_lay[32 * b4:32 * b4 + NK, :].rearrange(
                "p (g h) -> p g h", h=2)
            sap = a_f32[:]
            src = AP(sap.tensor, sap.offset + 2 * b4,
                     [list(sap.ap[0]), [8, 32], [1, 2]])
            nc.gpsimd.tensor_copy(dst, src)

        # biasT[2g+h, p] = bias_lay[p, 2g+h] via PE transpose; the -A_i
        # exp bias is then applied inside the PSUM accumulation by a
        # rank-1 matmul (stationary = biasT row, moving = ones_row), so
        # one Exp instruction covers both h halves.
        bias_bf = const_pool.tile([128, 64], BF16, tag="bias_bf",
                                  name="bias_bf")
        nc.vector.tensor_copy(bias_bf[:], bias_lay[:])
        pbt = psum_a.tile([64, 128], BF16, tag="pA", name="pbt")
        nc.tensor.transpose(pbt[:], bias_bf[:], s_tile[:, 960:1088])
        biasT = const_pool.tile([64, 128], BF16, tag="biasT", name="biasT")
        nc.vector.tensor_copy(biasT[:], pbt[:])
        # fold the 64 rows into partition 0 so each row is a legal
        # (base-partition-0) rank-1 stationary
        biasT_flat = const_pool.tile([1, 64, 128], BF16, tag="biasT_flat",
                                     name="biasT_flat")
        nc.sync.dma_start(out=biasT_flat[:], in_=biasT[:])
        ones_row = const_pool.tile([1, B], BF16, tag="ones_row",
                                   name="ones_row")
        nc.vector.memset(ones_row[:], 1.0)

        feat_own = const_pool.tile([128, 64], F32, tag="feat_own",
                                   name="feat_own")
        feat_mir = psum_m.tile([32, B], F32)

        # chunk-5 4-way i-merge: the 10 act rows replicated at partitions
        # {0,32,64,96} (rep5), a gathered per-(g,h) scalar matrix S54 with
        # S54[32*b4 + row, 2g + h] = act[640+row, 8g+2b4+h], and the
        # h-replicated A-value tile a2 for the -A_j correction matmul.
        rep5 = const_pool.tile([128, B], BF16, tag="rep5", name="rep5")
        nc.gpsimd.memset(rep5[:], 0.0)
        for b4 in range(4):
            nc.gpsimd.tensor_copy(rep5[32 * b4:32 * b4 + 10, :], tb5[0:10, :])
        s54 = const_pool.tile([128, 64], F32, tag="s54", name="s54")
        nc.gpsimd.memset(s54[:], 0.0)
        for b4 in range(4):
            dst = s54[32 * b4:32 * b4 + 10, :].rearrange(
                "p (g h) -> p g h", h=2)
            sap = act_f32[5][:]
            srcap = AP(sap.tensor, sap.offset + 2 * b4,
                       [list(sap.ap[0]), [8, 32], [1, 2]])
            nc.gpsimd.tensor_copy(dst, srcap)
        a2 = const_pool.tile([NK, 2, B], BF16, tag="a2", name="a2")
        t5 = tb5[AOFF:AOFF + NK, :]
        arows_bf = AP(t5.tensor, t5.offset, [list(t5.ap[0]), [0, 2], [1, B]])
        nc.gpsimd.tensor_copy(a2[:], arows_bf)

        # ---- Phase B (software-pipelined: exp one octet behind the
        # diffs/matmuls, own/mirror sums two behind, so no engine ever
        # head-of-line blocks on a cross-engine dependency) ----
        pl1_of = {}
        jt_of = {}

        def emit_exp(g):
            j0 = 8 * g
            jt = jt_pool.tile([128, 2, B], BF16, tag="jt", name="jt")
            nc.scalar.activation(
                jt[:, :, j0:],
                pl1_of.pop(g)[:, :, j0:],
                mybir.ActivationFunctionType.Exp,
                scale=-1.0,
            )
            jt_of[g] = jt

        def emit_sums(g):
            j0 = 8 * g
            JL = B - j0
            jt = jt_of.pop(g)
            jr = jt_pool.tile([128, 2, B], BF16, tag="jred", name="jred")
            for h in range(2):
                # own-sum via tensor_scalar accum_out (4x mode; the main
                # output is a dummy copy)
                nc.vector.tensor_scalar(
                    jr[:, h, j0:],
                    jt[:, h, j0:],
                    0.0,
                    0.0,
                    op0=mybir.AluOpType.add,
                    op1=mybir.AluOpType.add,
                    accum_out=feat_own[:, 2 * g + h:2 * g + h + 1],
                )
            if JL > 8:
                for h in range(2):
                    nc.tensor.matmul(
                        feat_mir[:, j0 + 8:],
                        s_tile[:, 384:416],
                        jt[:, h, j0 + 8:],
                        start=(g == 0 and h == 0),
                        stop=(g == 30 and h == 1),
                    )

        for g in range(32):
            j0 = 8 * g
            JL = B - j0
            pl1 = psum_b.tile([128, 2, B], F32)
            pl1_of[g] = pl1
            diffs = []
            for b4 in range(4):
                dts = []
                for t in range(NCH - 1):
                    dt_ = diff_pool.tile([CHUNKS[t][1], 2, B], BF16,
                                         tag=f"d{t}", name=f"d{t}")
                    dts.append(dt_)
                for t in range(NCH - 1):
                    for h in range(2):
                        i = 8 * g + 2 * b4 + h
                        msz = CHUNKS[t][1]
                        dst = dts[t][0:msz, h, j0:]
                        if _act_unit(t, g, b4, h):
                            nc.scalar.activation(
                                dst,
                                act_bf[t][0:msz, j0:],
                                mybir.ActivationFunctionType.Relu,
                                bias=act_neg[t][:, i:i + 1],
                                scale=1.0,
                            )
                        else:
                            nc.vector.tensor_scalar(
                                dst,
                                act_bf[t][0:msz, j0:],
                                act_f32[t][:, i:i + 1],
                                0.0,
                                op0=mybir.AluOpType.subtract,
                                op1=mybir.AluOpType.max,
                            )
                diffs.append(dts)
            # chunk-5 diffs: one 4-i-merged tensor_scalar per h
            d5m = diff_pool.tile([128, 2, B], BF16, tag="d5m", name="d5m")
            for h in range(2):
                nc.vector.tensor_scalar(
                    d5m[:, h, j0:],
                    rep5[:, j0:],
                    s54[:, 2 * g + h:2 * g + h + 1],
                    0.0,
                    op0=mybir.AluOpType.subtract,
                    op1=mybir.AluOpType.max,
                )
            # d-reduction on PE: chunks 0..4 col-tiled per b4, then the
            # merged chunk-5 diffs and the -A_j correction full-width.
            for t in range(NCH - 1):
                msz = CHUNKS[t][1]
                for b4 in range(4):
                    nc.tensor.matmul(
                        pl1[32 * b4:32 * b4 + 32, :, j0:],
                        s_tile[0:msz, 32 * t:32 * t + 32],
                        diffs[b4][t][0:msz, :, j0:],
                        start=(t == 0),
                        stop=False,
                        tile_position=(0, 32 * b4),
                    )
            for h in range(2):
                nc.tensor.matmul(
                    pl1[:, h, j0:],
                    s_tile[0:128, 448:576],
                    d5m[:, h, j0:],
                    start=False,
                    stop=False,
                )
                nc.tensor.matmul(
                    pl1[:, h, j0:],
                    s_tile[0:NK, 576:704],
                    a2[:, h, j0:],
                    start=False,
                    stop=False,
                )
                # + (-A_i) per partition: rank-1 (biasT row) x ones_row
                nc.tensor.matmul(
                    pl1[:, h, j0:],
                    biasT_flat[:, 2 * g + h, :],
                    ones_row[:, j0:],
                    start=False,
                    stop=(h == 1),
                )
            if g >= 1:
                emit_exp(g - 1)
            if g >= 2:
                emit_sums(g - 2)

        emit_exp(31)
        emit_sums(30)
        emit_sums(31)

        nc.sync.dma_start(out=fown_d[:], in_=feat_own[:])
        fmir_sb = const_pool.tile([32, B], F32, tag="fmir_sb", name="fmir_sb")
        nc.gpsimd.memset(fmir_sb[:, 0:8], 0.0)
        nc.vector.tensor_copy(fmir_sb[:, 8:], feat_mir[:, 8:])
        nc.sync.dma_start(out=fmir_d[:], in_=fmir_sb[:])
    nc.finalize()
    return nc


def _build_s_pack(w_core):
    s = np.zeros((128, SW), np.float32)
    q = np.arange(COLS)
    t = q // 128
    p = q % 128
    r = q // DK
    m5 = t < 5
    s[p[m5], 32 * t[m5] + r[m5]] = 2.0   # Sx2: 2*sum(relu), chunks 0..4
    for b4 in range(4):
        for rr in range(NK):
            s[32 * b4 + rr, 384 + rr] = 1.0  # M1: mirror partition-sum
    # MD: merged chunk-5 reduce (4 i's at partition offsets 32*b4)
    for b4 in range(4):
        for row in range(10):
            s[32 * b4 + row, 448 + 32 * b4 + 12] = 2.0
    # MA: -A_j correction for all four b4 row-groups
    for b4 in range(4):
        for rr in range(NK):
            s[rr, 576 + 32 * b4 + rr] = -1.0
    # WS: per-kernel column sums of this core's W (bf16-rounded, exactly
    # the values the PE would multiply), for the early A-table matmul
    ws = np.asarray(w_core, np.float32).reshape(IN_D, NK, DK).sum(axis=2)
    for k in range(8):
        s[:, 704 + 32 * k:704 + 32 * k + NK] = ws[128 * k:128 * (k + 1), :]
    s[np.arange(128), 960 + np.arange(128)] = 1.0   # I128 for PE transpose
    return s.astype(ml_dtypes.bfloat16)


_NC_CACHE = None


def _get_nc():
    global _NC_CACHE
    if _NC_CACHE is None:
        _NC_CACHE = build_nc()
    return _NC_CACHE


def make_in_maps(x, weight):
    x = np.asarray(x, np.float32)
    weight = np.asarray(weight, np.float32)
    xT = np.ascontiguousarray(x.T).astype(ml_dtypes.bfloat16)
    wp = np.zeros((IN_D, COLS * N_CORES), np.float32)
    wp[:, :weight.shape[1]] = weight
    maps = []
    for c in range(N_CORES):
        w_core = np.ascontiguousarray(
            wp[:, COLS * c:COLS * (c + 1)]).astype(ml_dtypes.bfloat16)
        maps.append({
            "xT": xT,
            "w": w_core,
            "s": _build_s_pack(w_core.astype(np.float32)),
        })
    return maps


def assemble(x, results):
    x = np.asarray(x, np.float32)
    feats = []
    for c in range(N_CORES):
        fo = np.asarray(results[c]["fown"], np.float32)   # [128, 64]
        fm = np.asarray(results[c]["fmir"], np.float32)   # [32, 256]
        F = fo.reshape(4, 32, 32, 2)[:, :NK]              # [b, r, g, h]
        own = F.transpose(2, 0, 3, 1).reshape(B, NK)
        feats.append(own + fm[:NK, :].T)
    features = np.concatenate(feats, axis=1)[:, :100]
    return np.concatenate([x, features], axis=1)


def kernel(x, weight):
    in_maps = make_in_maps(x, weight)
    nc = _get_nc()
    res = run_bass_kernel_spmd(nc, in_maps, list(range(N_CORES)))
    return assemble(x, res.results)


# revision 7
# speedup vs baseline: 1.0040x; 1.0040x over previous
"""MiniBatchDiscrimination Trainium2 kernel, v2 (triangular / symmetric).

Math (per reference):
    act = (x @ W).reshape(B, K, D)              # B=256, K=100, D=50
    l1[i,k,j] = sum_d |act[i,k,d] - act[j,k,d]|
    features[i,k] = sum_j exp(-l1[i,k,j])
    out = concat([x, features], axis=1)

Sharding: kernels K are sharded across 8 cores (13 per core, padded to
104).  Each core does the full pairwise work for its 13 kernels.

v2 exploits l1 symmetry: only pairs with j >= 8*floor(i/8) are computed.
For i-octet g (i = 8g+2b+h, b<4, h<2) the j-range is [8g, 256).  All
phase-B tiles use absolute-j columns: diff tiles are [rows, 2, 256]
with only cols [8g, 256) written per octet; the PE matmul streams the
[.., :, 8g:] slice.  l1 identity: |y| = 2 relu(y) - y, so
l1 = 2 R' - A_j + A_i with R'[i,j] = sum_d relu(a_jd - a_id) and
A[r, j] = sum_d act[(r,d), j].

Per-octet pipeline (software-pipelined: exp one octet behind, sums two):
  - relu diffs via DVE tensor_scalar (subtract, max vs 0; bf16 4x mode),
    10 of 48 units per octet on ScalarE (Relu, bias=-a_i) to balance
    engines; the 10-row chunk 5 is 4-i-merged (rows replicated at
    partitions {0,32,64,96}, gathered scalar matrix S54) so one
    instruction covers four i's.
  - PE reduce into pl1 = l1: 2.0 block-diag stationaries per b4 for
    chunks 0..4 (tile_position col groups), merged chunk-5 stationary,
    -I13 x a2 for -A_j, and a rank-1 (biasT row x ones) for +A_i.
    A itself comes from an early matmul A = WS.T @ xT (WS = host-side
    per-kernel column sums of W, shipped in the s-pack); biasT is the
    A-layout table transposed on PE and folded to partition 0 by an
    SBUF->SBUF DMA.
  - exp: one ScalarE Exp(-pl1) per octet -> jt = exp(-l1) exactly
    (diagonal exactly 1; all A terms trace to one bf16 table).
  - own-sum: per-h tensor_scalar with accum_out (4x mode).
  - mirror-sum: ones-stationary PE matmul accumulating
    sum_{i in octet} jt[i, j] into persistent PSUM feat_mir[r, j] for
    j >= 8g+8 (within-octet pairs are covered by the own-sums).
Host: feature[m, r] = own[m, r] + mir[r, m]; concat with x (exact).
"""

import numpy as np
import ml_dtypes
from contextlib import ExitStack

import concourse.bass as bass
import concourse.bacc as bacc
import concourse.tile as tile
from concourse import mybir
from concourse.ap import AP
from concourse.bass_utils import run_bass_kernel_spmd

B = 256          # batch
IN_D = 1024      # input dim
NK = 13          # kernels per core (8*13 = 104 >= 100)
DK = 50          # dim per kernel
COLS = NK * DK   # 650 act_T rows per core
N_CORES = 8
CHUNKS = [(0, 128), (128, 128), (256, 128), (384, 128), (512, 128), (640, 10)]
NCH = len(CHUNKS)
AOFF = 32        # partition offset of the A-rows in the combined tiles
SW = 1088        # s-pack width

F32 = mybir.dt.float32
BF16 = mybir.dt.bfloat16


def _act_unit(t, g, b4, h):
    """True if this diff unit runs on ScalarE (Activation engine).

    Per-octet balance: ScalarE takes 11 of the 48 units each octet
    (unit costs: ScalarE ~185+0.833*JL ns vs DVE ~60+0.26*JL ns, and
    ScalarE also runs the two Exp instructions)."""
    return t == 0 or (t == 1 and 2 * b4 + h < 2)


def build_nc():
    nc = bacc.Bacc()
    xT_d = nc.declare_dram_parameter("xT", [IN_D, B], BF16, isOutput=False)
    w_d = nc.declare_dram_parameter("w", [IN_D, COLS], BF16, isOutput=False)
    s_d = nc.declare_dram_parameter("s", [128, SW], BF16, isOutput=False)
    fown_d = nc.declare_dram_parameter("fown", [128, 64], F32, isOutput=True)
    fmir_d = nc.declare_dram_parameter("fmir", [32, B], F32, isOutput=True)

    with ExitStack() as ctx:
        tc = ctx.enter_context(tile.TileContext(nc))
        const_pool = ctx.enter_context(tc.tile_pool(name="const", bufs=1))
        psum_a = ctx.enter_context(tc.tile_pool(name="psum_a", bufs=1, space="PSUM"))
        psum_b = ctx.enter_context(tc.tile_pool(name="psum_b", bufs=3, space="PSUM"))
        psum_m = ctx.enter_context(tc.tile_pool(name="psum_m", bufs=1, space="PSUM"))
        diff_pool = ctx.enter_context(tc.tile_pool(name="diff", bufs=8))
        jt_pool = ctx.enter_context(tc.tile_pool(name="jt", bufs=4))

        # ---- load inputs; W in two column halves so phase A's first
        # half (chunks 0-2) starts after ~1/2 of the W bytes arrive ----
        WSPLIT = 384
        xt_all = const_pool.tile([128, 8, B], BF16, tag="xt", name="xt_all")
        for kh in range(2):
            nc.sync.dma_start(
                out=xt_all[:, 4 * kh:4 * kh + 4, :],
                in_=AP(xT_d, 128 * B * 4 * kh,
                       [[B, 128], [128 * B, 4], [1, B]]))
        xt_tiles = [xt_all[:, k, :] for k in range(8)]
        s_tile = const_pool.tile([128, SW], BF16, tag="s", name="s_tile")
        nc.sync.dma_start(out=s_tile[:], in_=s_d[:])
        wl_all = const_pool.tile([128, 8, WSPLIT], BF16, tag="wl",
                                 name="wl_all")
        for kh in range(2):
            nc.sync.dma_start(
                out=wl_all[:, 4 * kh:4 * kh + 4, :],
                in_=AP(w_d, 128 * COLS * 4 * kh,
                       [[COLS, 128], [128 * COLS, 4], [1, WSPLIT]]))
        w_lo = [wl_all[:, k, :] for k in range(8)]
        wh_all = const_pool.tile([128, 8, COLS - WSPLIT], BF16, tag="wh",
                                 name="wh_all")
        for kh in range(2):
            nc.sync.dma_start(
                out=wh_all[:, 4 * kh:4 * kh + 4, :],
                in_=AP(w_d, WSPLIT + 128 * COLS * 4 * kh,
                       [[COLS, 128], [128 * COLS, 4], [1, COLS - WSPLIT]]))
        w_hi = [wh_all[:, k, :] for k in range(8)]

        # ---- Phase A: act_T = W.T @ xT (k outer: overlap DMA/PE) ----
        act_bf = []    # bf16 tiles; act_bf[5] is the combined tile
        act_f32 = []   # fp32 upcasts (DVE tensor_scalar scalar operands)
        act_neg = {}   # negated fp32 (ScalarE relu bias)
        tb5 = const_pool.tile([AOFF + NK, B], BF16, tag="actb5", name="tb5")
        nc.gpsimd.memset(tb5[:], 0.0)
        pA = None
        pa_tiles = {}
        for half in range(2):
            ts = range(3 * half, 3 * half + 3)
            for t in ts:
                pa_tiles[t] = psum_a.tile([CHUNKS[t][1], B], F32,
                                          tag=f"pa{t % 3}", name=f"pa{t}")
            for t in ts:
                for k in range(8):
                    mstart, msz = CHUNKS[t]
                    if half == 0:
                        wsl = w_lo[k][:, mstart:mstart + msz]
                    else:
                        wsl = w_hi[k][:, mstart - WSPLIT:mstart - WSPLIT + msz]
                    nc.tensor.matmul(
                        pa_tiles[t][:],
                        wsl,
                        xt_tiles[k][:],
                        start=(k == 0),
                        stop=(k == 7),
                    )
                if half == 0 and t == 0:
                    # A-table: A = WS.T @ xT (WS = host-precomputed column
                    # sums of W in the s-pack).  After chunk 0 so Act/DVE
                    # unblock first; done long before its consumers.
                    pA = psum_a.tile([32, B], F32, name="pA")
                    for k in range(8):
                        nc.tensor.matmul(
                            pA[:],
                            s_tile[0:128, 704 + 32 * k:704 + 32 * k + 32],
                            xt_tiles[k][:],
                            start=(k == 0),
                            stop=(k == 7),
                        )
        for t, (mstart, msz) in enumerate(CHUNKS):
            pa = pa_tiles[t]
            if t < NCH - 1:
                tb = const_pool.tile([msz, B], BF16, tag=f"actb{t}",
                                     name=f"actb{t}")
                nc.vector.tensor_copy(tb[:], pa[:])
                src = tb[:]
                act_bf.append(tb)
            else:
                nc.vector.tensor_copy(tb5[0:10, :], pa[:])
                src = tb5[0:10, :]
                act_bf.append(tb5)
            tf = const_pool.tile([msz, B], F32, tag=f"actf{t}", name=f"actf{t}")
            nc.gpsimd.tensor_copy(tf[:], src)
            act_f32.append(tf)
            if any(_act_unit(t, g, b4, h) for g in range(32)
                   for b4 in range(4) for h in range(2)):
                tn = const_pool.tile([msz, B], F32, tag=f"actn{t}",
                                     name=f"actn{t}")
                nc.scalar.mul(tn[:], src, -1.0)
                act_neg[t] = tn

        # A values -> rows AOFF.. of tb5 (bf16)
        nc.vector.tensor_copy(tb5[AOFF:AOFF + NK, :], pA[0:NK, :])
        a_f32 = const_pool.tile([NK, B], F32, tag="a_f32", name="a_f32")
        nc.gpsimd.tensor_copy(a_f32[:], tb5[AOFF:AOFF + NK, :])

        # A_i layout: bias_lay[32b + r, 2g + h] = +A[r, 8g + 2b + h]
        # (added INTO pl1 by a rank-1 matmul; l1 = 2R' - A_j + A_i)
        bias_lay = const_pool.tile([128, 64], F32, tag="bias_lay",
                                   name="bias_lay")
        nc.gpsimd.memset(bias_lay[:], 0.0)
        for b4 in range(4):
            dst = bias_lay[32 * b4:32 * b4 + NK, :].rearrange(
                "p (g h) -> p g h", h=2)
            sap = a_f32[:]
            src = AP(sap.tensor, sap.offset + 2 * b4,
                     [list(sap.ap[0]), [8, 32], [1, 2]])
            nc.gpsimd.tensor_copy(dst, src)

        # biasT[2g+h, p] = bias_lay[p, 2g+h] via PE transpose; the -A_i
        # exp bias is then applied inside the PSUM accumulation by a
        # rank-1 matmul (stationary = biasT row, moving = ones_row), so
        # one Exp instruction covers both h halves.
        bias_bf = const_pool.tile([128, 64], BF16, tag="bias_bf",
                                  name="bias_bf")
        nc.vector.tensor_copy(bias_bf[:], bias_lay[:])
        pbt = psum_a.tile([64, 128], BF16, tag="pA", name="pbt")
        nc.tensor.transpose(pbt[:], bias_bf[:], s_tile[:, 960:1088])
        biasT = const_pool.tile([64, 128], BF16, tag="biasT", name="biasT")
        nc.vector.tensor_copy(biasT[:], pbt[:])
        # fold the 64 rows into partition 0 so each row is a legal
        # (base-partition-0) rank-1 stationary
        biasT_flat = const_pool.tile([1, 64, 128], BF16, tag="biasT_flat",
                                     name="biasT_flat")
        nc.sync.dma_start(out=biasT_flat[:], in_=biasT[:])
        ones_row = const_pool.tile([1, B], BF16, tag="ones_row",
                                   name="ones_row")
        nc.vector.memset(ones_row[:], 1.0)

        feat_own = const_pool.tile([128, 64], F32, tag="feat_own",
                                   name="feat_own")
        feat_mir = psum_m.tile([32, B], F32)

        # chunk-5 4-way i-merge: the 10 act rows replicated at partitions
        # {0,32,64,96} (rep5), a gathered per-(g,h) scalar matrix S54 with
        # S54[32*b4 + row, 2g + h] = act[640+row, 8g+2b4+h], and the
        # h-replicated A-value tile a2 for the -A_j correction matmul.
        rep5 = const_pool.tile([128, B], BF16, tag="rep5", name="rep5")
        nc.gpsimd.memset(rep5[:], 0.0)
        for b4 in range(4):
            nc.gpsimd.tensor_copy(rep5[32 * b4:32 * b4 + 10, :], tb5[0:10, :])
        s54 = const_pool.tile([128, 64], F32, tag="s54", name="s54")
        nc.gpsimd.memset(s54[:], 0.0)
        for b4 in range(4):
            dst = s54[32 * b4:32 * b4 + 10, :].rearrange(
                "p (g h) -> p g h", h=2)
            sap = act_f32[5][:]
            srcap = AP(sap.tensor, sap.offset + 2 * b4,
                       [list(sap.ap[0]), [8, 32], [1, 2]])
            nc.gpsimd.tensor_copy(dst, srcap)
        a2 = const_pool.tile([NK, 2, B], BF16, tag="a2", name="a2")
        t5 = tb5[AOFF:AOFF + NK, :]
        arows_bf = AP(t5.tensor, t5.offset, [list(t5.ap[0]), [0, 2], [1, B]])
        nc.gpsimd.tensor_copy(a2[:], arows_bf)

        # ---- Phase B (software-pipelined: exp one octet behind the
        # diffs/matmuls, own/mirror sums two behind, so no engine ever
        # head-of-line blocks on a cross-engine dependency) ----
        pl1_of = {}
        jt_of = {}

        def emit_exp(g):
            j0 = 8 * g
            jt = jt_pool.tile([128, 2, B], BF16, tag="jt", name="jt")
            nc.scalar.activation(
                jt[:, :, j0:],
                pl1_of.pop(g)[:, :, j0:],
                mybir.ActivationFunctionType.Exp,
                scale=-1.0,
            )
            jt_of[g] = jt

        def emit_sums(g):
            j0 = 8 * g
            JL = B - j0
            jt = jt_of.pop(g)
            jr = jt_pool.tile([128, 2, B], BF16, tag="jred", name="jred")
            for h in range(2):
                # own-sum via tensor_scalar accum_out (4x mode; the main
                # output is a dummy copy)
                nc.vector.tensor_scalar(
                    jr[:, h, j0:],
                    jt[:, h, j0:],
                    0.0,
                    0.0,
                    op0=mybir.AluOpType.add,
                    op1=mybir.AluOpType.add,
                    accum_out=feat_own[:, 2 * g + h:2 * g + h + 1],
                )
            if JL > 8:
                for h in range(2):
                    nc.tensor.matmul(
                        feat_mir[:, j0 + 8:],
                        s_tile[:, 384:416],
                        jt[:, h, j0 + 8:],
                        start=(g == 0 and h == 0),
                        stop=(g == 30 and h == 1),
                    )

        for g in range(32):
            j0 = 8 * g
            JL = B - j0
            pl1 = psum_b.tile([128, 2, B], F32)
            pl1_of[g] = pl1
            diffs = []
            for b4 in range(4):
                dts = []
                for t in range(NCH - 1):
                    dt_ = diff_pool.tile([CHUNKS[t][1], 2, B], BF16,
                                         tag=f"d{t}", name=f"d{t}")
                    dts.append(dt_)
                for t in range(NCH - 1):
                    for h in range(2):
                        i = 8 * g + 2 * b4 + h
                        msz = CHUNKS[t][1]
                        dst = dts[t][0:msz, h, j0:]
                        if _act_unit(t, g, b4, h):
                            nc.scalar.activation(
                                dst,
                                act_bf[t][0:msz, j0:],
                                mybir.ActivationFunctionType.Relu,
                                bias=act_neg[t][:, i:i + 1],
                                scale=1.0,
                            )
                        else:
                            nc.vector.tensor_scalar(
                                dst,
                                act_bf[t][0:msz, j0:],
                                act_f32[t][:, i:i + 1],
                                0.0,
                                op0=mybir.AluOpType.subtract,
                                op1=mybir.AluOpType.max,
                            )
                diffs.append(dts)
            # chunk-5 diffs: one 4-i-merged tensor_scalar per h
            d5m = diff_pool.tile([128, 2, B], BF16, tag="d5m", name="d5m")
            for h in range(2):
                nc.vector.tensor_scalar(
                    d5m[:, h, j0:],
                    rep5[:, j0:],
                    s54[:, 2 * g + h:2 * g + h + 1],
                    0.0,
                    op0=mybir.AluOpType.subtract,
                    op1=mybir.AluOpType.max,
                )
            # d-reduction on PE: chunks 0..4 col-tiled per b4, then the
            # merged chunk-5 diffs and the -A_j correction full-width.
            for t in range(NCH - 1):
                msz = CHUNKS[t][1]
                for b4 in range(4):
                    nc.tensor.matmul(
                        pl1[32 * b4:32 * b4 + 32, :, j0:],
                        s_tile[0:msz, 32 * t:32 * t + 32],
                        diffs[b4][t][0:msz, :, j0:],
                        start=(t == 0),
                        stop=False,
                        tile_position=(0, 32 * b4),
                    )
            for h in range(2):
                nc.tensor.matmul(
                    pl1[:, h, j0:],
                    s_tile[0:128, 448:576],
                    d5m[:, h, j0:],
                    start=False,
                    stop=False,
                )
                nc.tensor.matmul(
                    pl1[:, h, j0:],
                    s_tile[0:NK, 576:704],
                    a2[:, h, j0:],
                    start=False,
                    stop=False,
                )
                # + (-A_i) per partition: rank-1 (biasT row) x ones_row
                nc.tensor.matmul(
                    pl1[:, h, j0:],
                    biasT_flat[:, 2 * g + h, :],
                    ones_row[:, j0:],
                    start=False,
                    stop=(h == 1),
                )
            if g >= 1:
                emit_exp(g - 1)
            if g >= 2:
                emit_sums(g - 2)
            if g == 18:
                # first half of feat_own (octets 0..15) is final; ship it
                nc.sync.dma_start(out=fown_d[:, 0:32], in_=feat_own[:, 0:32])

        emit_exp(31)
        emit_sums(30)
        emit_sums(31)

        nc.sync.dma_start(out=fown_d[:, 32:], in_=feat_own[:, 32:])
        fmir_sb = const_pool.tile([32, B], F32, tag="fmir_sb", name="fmir_sb")
        nc.gpsimd.memset(fmir_sb[:, 0:8], 0.0)
        nc.vector.tensor_copy(fmir_sb[:, 8:], feat_mir[:, 8:])
        nc.sync.dma_start(out=fmir_d[:], in_=fmir_sb[:])
    nc.finalize()
    return nc


def _build_s_pack(w_core):
    s = np.zeros((128, SW), np.float32)
    q = np.arange(COLS)
    t = q // 128
    p = q % 128
    r = q // DK
    m5 = t < 5
    s[p[m5], 32 * t[m5] + r[m5]] = 2.0   # Sx2: 2*sum(relu), chunks 0..4
    for b4 in range(4):
        for rr in range(NK):
            s[32 * b4 + rr, 384 + rr] = 1.0  # M1: mirror partition-sum
    # MD: merged chunk-5 reduce (4 i's at partition offsets 32*b4)
    for b4 in range(4):
        for row in range(10):
            s[32 * b4 + row, 448 + 32 * b4 + 12] = 2.0
    # MA: -A_j correction for all four b4 row-groups
    for b4 in range(4):
        for rr in range(NK):
            s[rr, 576 + 32 * b4 + rr] = -1.0
    # WS: per-kernel column sums of this core's W (bf16-rounded, exactly
    # the values the PE would multiply), for the early A-table matmul
    ws = np.asarray(w_core, np.float32).reshape(IN_D, NK, DK).sum(axis=2)
    for k in range(8):
        s[:, 704 + 32 * k:704 + 32 * k + NK] = ws[128 * k:128 * (k + 1), :]
    s[np.arange(128), 960 + np.arange(128)] = 1.0   # I128 for PE transpose
    return s.astype(ml_dtypes.bfloat16)


_NC_CACHE = None


def _get_nc():
    global _NC_CACHE
    if _NC_CACHE is None:
        _NC_CACHE = build_nc()
    return _NC_CACHE


def make_in_maps(x, weight):
    x = np.asarray(x, np.float32)
    weight = np.asarray(weight, np.float32)
    xT = np.ascontiguousarray(x.T).astype(ml_dtypes.bfloat16)
    wp = np.zeros((IN_D, COLS * N_CORES), np.float32)
    wp[:, :weight.shape[1]] = weight
    maps = []
    for c in range(N_CORES):
        w_core = np.ascontiguousarray(
            wp[:, COLS * c:COLS * (c + 1)]).astype(ml_dtypes.bfloat16)
        maps.append({
            "xT": xT,
            "w": w_core,
            "s": _build_s_pack(w_core.astype(np.float32)),
        })
    return maps


def assemble(x, results):
    x = np.asarray(x, np.float32)
    feats = []
    for c in range(N_CORES):
        fo = np.asarray(results[c]["fown"], np.float32)   # [128, 64]
        fm = np.asarray(results[c]["fmir"], np.float32)   # [32, 256]
        F = fo.reshape(4, 32, 32, 2)[:, :NK]              # [b, r, g, h]
        own = F.transpose(2, 0, 3, 1).reshape(B, NK)
        feats.append(own + fm[:NK, :].T)
    features = np.concatenate(feats, axis=1)[:, :100]
    return np.concatenate([x, features], axis=1)


def kernel(x, weight):
    in_maps = make_in_maps(x, weight)
    nc = _get_nc()
    res = run_bass_kernel_spmd(nc, in_maps, list(range(N_CORES)))
    return assemble(x, res.results)
